# revision 33
# baseline (speedup 1.0000x reference)
"""Trainium2 Bass kernel for nn_Connection_v5extend (8-core data-parallel).

kernel(**inputs) takes the FULL unsharded inputs (as produced by
setup_inputs) and returns the FULL [4096, 256] float32 output.

Math (per core, B=512 rows, D=128):
    x, v  = input[:, :D], input[:, D:]
    h     = W1 x + b1;  a = relu(h);  mask = (h > 0)
    s     = sigmoid(W2 a + b2);  sig = s (1 - s);  nsig = -sig
    p'    = v^2 * nsig                     (sgn folded into W2s = sgn.W2)
    u'    = W2s^T p' = -W2^T (v^2 sgn sig);  r' = mask * u';  t1' = -W1^T r
    t2    = W2 (mask * (W1 v))
    a2    = (2 v sig) t2;  cc = sgn*t1' + a2 = 2 v sig t2 - sgn t1
    dv    = cc / (s + CONST)
    out   = concat([v, dv], axis=1)

Implementation notes: feature-major on chip (features on partitions,
batch on the free dim); batch rows permuted (row = 4p + c) so each
partition loads one contiguous 2KB block per input DMA (the output
store undoes the permutation); W1 uses a row-parity split (row = 2p+h)
consistently across every k-contraction, which lets W1/W2 load with
large descriptors and b1 spread to partitions with tiny PE transposes;
sgn is folded into the mm_u stationary (w2s = sgn * W2) so no sign
masks appear in the elementwise chain; low-priority warm-up matmuls
ramp the PE clock during the input-DMA wait; the v-passthrough is
written from SBUF after the input lands; elementwise work is spread
across ACT (relu/sigmoid/PSUM drains), DVE (all PSUM-sourced tensor
ops, reciprocal), and Pool/GpSimd (SBUF-only products).
"""

import sys

sys.path.insert(0, "/opt/trn_rl_repo")

import numpy as np

import concourse.bass as bass  # noqa: F401
import concourse.bacc as bacc
import concourse.mybir as mybir
import concourse.tile as tile
from concourse.masks import make_identity
from concourse.bass_utils import run_bass_kernel_spmd

F32 = mybir.dt.float32
F32R = mybir.dt.float32r
AF = mybir.ActivationFunctionType
ALU = mybir.AluOpType

D = 128
CONST = 0.618
SIGN = 4
N_CORES = 8
BATCH = 4096
B = BATCH // N_CORES  # rows per core
NCH = B // D          # 4 batch chunks of 128
SEG = B // 2          # 256 cols per pipeline segment
N_DUM = 8             # warm-up matmuls


def _build(nc):
    inp = nc.dram_tensor("inp", [B, 2 * D], F32, kind="ExternalInput").ap()
    W1 = nc.dram_tensor("W1", [2 * D, D], F32, kind="ExternalInput").ap()
    b1 = nc.dram_tensor("b1", [2 * D], F32, kind="ExternalInput").ap()
    W2 = nc.dram_tensor("W2", [D, 2 * D], F32, kind="ExternalInput").ap()
    b2 = nc.dram_tensor("b2", [D], F32, kind="ExternalInput").ap()
    out = nc.dram_tensor("out", [B, 2 * D], F32, kind="ExternalOutput").ap()

    def gsl(g):
        return slice(g * SEG, (g + 1) * SEG)

    with tile.TileContext(nc) as tc:
        with (
            tc.tile_pool(name="cst", bufs=1) as cst,
            tc.tile_pool(name="sb", bufs=1) as sb,
            tc.tile_pool(name="pblk", bufs=4, space="PSUM") as pblk,
            tc.tile_pool(name="pmis", bufs=1, space="PSUM") as pmis,
            tc.tile_pool(name="pu", bufs=1, space="PSUM") as pu,
            tc.tile_pool(name="pt1", bufs=1, space="PSUM") as pt1,
            tc.tile_pool(name="pt2", bufs=1, space="PSUM") as pt2,
        ):
            # ---------------- DMA issues (first thing on each queue) ------
            it = sb.tile([D, NCH, 2 * D], F32, tag="it", name="it")
            inr = inp.rearrange("(p c) f -> p c f", c=NCH)
            # x columns land first (they gate the transpose->mm_h chain)
            nc.scalar.dma_start(it[:, 0:2, 0:D], inr[:, 0:2, 0:D])
            nc.scalar.dma_start(it[:, 0:2, D:2 * D], inr[:, 0:2, D:2 * D])
            b1row = cst.tile([1, 2 * D], F32, tag="b1row", name="b1row")
            nc.scalar.dma_start(b1row[:], b1.rearrange("(o k) -> o k", o=1))
            b2row = cst.tile([1, D], F32, tag="b2row", name="b2row")
            nc.scalar.dma_start(b2row[:], b2.rearrange("(o k) -> o k", o=1))

            w1k = cst.tile([D, 2, D], F32, tag="w1k", name="w1k")
            nc.sync.dma_start(w1k[:], W1.rearrange("(p h) j -> p h j", h=2))
            nc.sync.dma_start(it[:, 2:4, 0:D], inr[:, 2:4, 0:D])
            nc.sync.dma_start(it[:, 2:4, D:2 * D], inr[:, 2:4, D:2 * D])
            w2n = cst.tile([D, 2 * D], F32, tag="w2n", name="w2n")
            nc.sync.dma_start(w2n[:], W2)
            # v passthrough from SBUF (naturally ordered after input DMAs)
            nc.sync.dma_start(
                out.rearrange("(p c) f -> p c f", c=NCH)[:, :, 0:D],
                it[:, :, D:2 * D])

            # ---------------- constants -----------------------------------
            zer = cst.tile([D, 2 * D], F32, tag="zer", name="zer")
            nc.vector.memset(zer[:], 0.0)
            # sgnc = sgn  (-1 for i < SIGN, +1 otherwise)
            sgnc = cst.tile([D, 1], F32, tag="sgnc", name="sgnc")
            nc.vector.memset(sgnc[:], 1.0)
            nc.vector.memset(sgnc[:SIGN, :], -1.0)
            dum = cst.tile([D, 2 * D], F32R, tag="dum", name="dum")
            nc.vector.tensor_copy(dum[:], zer[:])

            ident = cst.tile([D, D], F32, tag="ident", name="ident")
            make_identity(nc, ident[:])

            # prime the {sigmoid,relu,identity,copy} activation table set
            warmt = cst.tile([D, 1], F32, tag="warmt", name="warmt")
            nc.scalar.activation(warmt[:], sgnc[:, 0:1], AF.Sigmoid)

            # ---------------- bias spread (tiny PE transposes) ------------
            # b1c[p, h] = b1[2p + h] matching the W1 row-parity split
            bias_ps = pmis.tile([D, 3], F32, tag="mis", name="mis")
            b1v = b1row[0:1, :].rearrange("o (p h) -> o h p", h=2)
            nc.tensor.transpose(bias_ps[:, 0:1], b1v[:, 0, :], ident[0:1, 0:1])
            nc.tensor.transpose(bias_ps[:, 1:2], b1v[:, 1, :], ident[0:1, 0:1])
            nc.tensor.transpose(bias_ps[:, 2:3], b2row[0:1, :],
                                ident[0:1, 0:1])
            bcols = cst.tile([D, 3], F32, tag="bcols", name="bcols")
            nc.vector.tensor_copy(bcols[:], bias_ps[:])

            # ---------------- weight prep ---------------------------------
            # w1T[:, h, :] = W1 parity-half h, transposed (PE + scalar copy)
            w1T = cst.tile([D, 2, D], F32R, tag="w1T", name="w1T")
            w1T_ps = pmis.tile([D, 2, D], F32, tag="mis", name="mis")
            nc.tensor.transpose(w1T_ps[:, 0, :], w1k[:, 0, :], ident[:])
            nc.tensor.transpose(w1T_ps[:, 1, :], w1k[:, 1, :], ident[:])
            nc.scalar.copy(w1T[:], w1T_ps[:])

            # w2s = sgn-row-scaled W2 (f32r), stationary for mm_u
            # (the scalar op is issued later, after the first relus)
            w2s = cst.tile([D, 2 * D], F32R, tag="w2s", name="w2s")
            w2ur = w2s[:].rearrange("i (k h) -> i h k", h=2)

            # w1kr: f32r copy of W1 (stationary for mm_t1)
            w1kr = cst.tile([D, 2, D], F32R, tag="w1kr", name="w1kr")
            nc.vector.tensor_copy(w1kr[:], w1k[:])

            # w2T[:, h, :] = transpose of W2 columns with parity h
            w2v = w2n[:].rearrange("i (k h) -> i h k", h=2)
            w2T = cst.tile([D, 2, D], F32R, tag="w2T", name="w2T")
            w2T_ps = pmis.tile([D, 2, D], F32, tag="mis", name="mis")
            nc.tensor.transpose(w2T_ps[:, 0, :], w2v[:, 0, :], ident[:])
            nc.tensor.transpose(w2T_ps[:, 1, :], w2v[:, 1, :], ident[:])

            def s_w2prep():
                nc.scalar.activation(w2s[:], w2n[:], AF.Copy,
                                     scale=sgnc[:, 0:1])
                nc.scalar.copy(w2T[:], w2T_ps[:])

            # ---------------- PE warm-up ----------------------------------
            # independent matmuls on zeros keep the PE busy (HAM clock ramp)
            # during the input-DMA wait; low priority so real PE work
            # preempts them as its inputs land.  The final copy into dum
            # (value-preserving: 0.0) gives pd a reader so DCE keeps them.
            pd = pu.tile([D, 2 * D], F32, tag="u", name="u")
            for i in range(N_DUM):
                nc.tensor.matmul(pd[:], dum[:, 0:D], dum[:])
            nc.vector.tensor_copy(dum[:, 0:1], pd[:, 0:1])

            # ---------------- data tiles ----------------------------------
            # xv[:, 0, c, :] = x chunk c transposed; [:, 1, c, :] = v
            xv = sb.tile([D, 2, NCH, D], F32R, tag="xv", name="xv")
            a = sb.tile([D, 2, B], F32R, tag="a", name="a")
            mw = sb.tile([D, 2, B], F32R, tag="mwt", name="mwt")
            rr = sb.tile([D, 2, B], F32R, tag="rr", name="rr")
            s_ = sb.tile([D, B], F32, tag="s", name="s")
            nsig = sb.tile([D, B], F32, tag="nsig", name="nsig")
            vsq = sb.tile([D, B], F32, tag="vsq", name="vsq")
            p_ = sb.tile([D, B], F32R, tag="p", name="p")
            rp = sb.tile([D, B], F32, tag="rp", name="rp")
            rec = sb.tile([D, B], F32, tag="rec", name="rec")
            vs = sb.tile([D, B], F32, tag="vs", name="vs")
            nv2 = sb.tile([D, B], F32, tag="nv2", name="nv2")
            a2 = sb.tile([D, B], F32, tag="a2", name="a2")
            cc = sb.tile([D, B], F32, tag="cc", name="cc")
            dv = sb.tile([D, B], F32, tag="dv", name="dv")
            ot = sb.tile([D, NCH, D], F32, tag="ot", name="ot")

            vTf = xv[:, 1, :, :].rearrange("p c d -> p (c d)").bitcast(F32)

            pxv = [None, None]
            hp = [None, None]
            wp = [None, None]
            zp = [None, None]
            up = [None, None]
            t1p = [None, None]
            t2p = [None, None]

            # ---------------- stage helpers -------------------------------
            def tin_x(g):
                pxv[g] = pblk.tile([D, 2, 2, D], F32, tag="blk", name="blk")
                for k in range(2):
                    c = 2 * g + k
                    nc.tensor.transpose(pxv[g][:, 0, k, :], it[:, c, 0:D],
                                        ident[:])

            def tin_v(g):
                for k in range(2):
                    c = 2 * g + k
                    nc.tensor.transpose(pxv[g][:, 1, k, :], it[:, c, D:2 * D],
                                        ident[:])

            def s_xcopy(g):
                nc.scalar.copy(xv[:, 0, 2 * g:2 * g + 2, :], pxv[g][:, 0, :, :])

            def v_vcopy(g):
                nc.vector.tensor_copy(xv[:, 1, 2 * g:2 * g + 2, :],
                                      pxv[g][:, 1, :, :])

            def xT(g):
                return xv[:, 0, 2 * g:2 * g + 2, :].rearrange(
                    "p c d -> p (c d)")

            def vT(g):
                return xv[:, 1, 2 * g:2 * g + 2, :].rearrange(
                    "p c d -> p (c d)")

            def mm_h(g):
                hp[g] = pblk.tile([D, 2, SEG], F32, tag="blk", name="blk")
                nc.tensor.matmul(hp[g][:, 0, :], w1T[:, 0, :], xT(g))
                nc.tensor.matmul(hp[g][:, 1, :], w1T[:, 1, :], xT(g))

            def mm_w(g):
                wp[g] = pblk.tile([D, 2, SEG], F32, tag="blk", name="blk")
                nc.tensor.matmul(wp[g][:, 0, :], w1T[:, 0, :], vT(g))
                nc.tensor.matmul(wp[g][:, 1, :], w1T[:, 1, :], vT(g))

            def mm_z(g):
                zp[g] = pmis.tile([D, SEG], F32, tag="mis", name="mis")
                nc.tensor.matmul(zp[g][:], w2T[:, 0, :], a[:, 0, gsl(g)],
                                 start=True, stop=False)
                nc.tensor.matmul(zp[g][:], w2T[:, 1, :], a[:, 1, gsl(g)],
                                 start=False, stop=True)

            def mm_u(g):
                up[g] = pu.tile([D, 2, SEG], F32, tag="u", name="u")
                nc.tensor.matmul(up[g][:, 0, :], w2ur[:, 0, :], p_[:, gsl(g)])
                nc.tensor.matmul(up[g][:, 1, :], w2ur[:, 1, :], p_[:, gsl(g)])

            def mm_t1(g):
                t1p[g] = pt1.tile([D, SEG], F32, tag="t1", name="t1")
                nc.tensor.matmul(t1p[g][:], w1kr[:, 0, :], rr[:, 0, gsl(g)],
                                 start=True, stop=False)
                nc.tensor.matmul(t1p[g][:], w1kr[:, 1, :], rr[:, 1, gsl(g)],
                                 start=False, stop=True)

            def mm_t2(g):
                t2p[g] = pt2.tile([D, SEG], F32, tag="t2", name="t2")
                nc.tensor.matmul(t2p[g][:], w2T[:, 0, :], mw[:, 0, gsl(g)],
                                 start=True, stop=False)
                nc.tensor.matmul(t2p[g][:], w2T[:, 1, :], mw[:, 1, gsl(g)],
                                 start=False, stop=True)

            # elementwise stages
            def s_relu(g, h):
                nc.scalar.activation(a[:, h, gsl(g)], hp[g][:, h, :], AF.Relu,
                                     bias=bcols[:, h:h + 1])

            def s_sig(g):
                nc.scalar.activation(s_[:, gsl(g)], zp[g][:], AF.Sigmoid,
                                     bias=bcols[:, 2:3])

            def v_nv2():
                # nv2 = -2v
                nc.vector.tensor_single_scalar(nv2[:], vTf, -2.0, ALU.mult)

            def v_vsq():
                nc.vector.tensor_mul(vsq[:], vTf, vTf)

            def v_nsig(g):
                # nsig = (s - 1) * s = -s(1-s)
                nc.vector.scalar_tensor_tensor(
                    nsig[:, gsl(g)], s_[:, gsl(g)], 1.0, s_[:, gsl(g)],
                    ALU.subtract, ALU.mult)

            def g_p(g):
                # p' = v^2 * nsig = -v^2 sig  (sgn lives in w2s)
                nc.gpsimd.tensor_mul(p_[:, gsl(g)], vsq[:, gsl(g)],
                                     nsig[:, gsl(g)])

            def v_rp(g):
                nc.vector.tensor_single_scalar(
                    rp[:, gsl(g)], s_[:, gsl(g)], CONST, ALU.add)

            def g_vs(g):
                # vs = (-2v) * nsig = 2 v sig
                nc.gpsimd.tensor_mul(vs[:, gsl(g)], nv2[:, gsl(g)],
                                     nsig[:, gsl(g)])

            def g_dv(g):
                nc.gpsimd.tensor_mul(dv[:, gsl(g)], cc[:, gsl(g)],
                                     rec[:, gsl(g)])

            def v_mw(g):
                # mw = (a > 0) * (W1 v)
                nc.vector.scalar_tensor_tensor(
                    mw[:, :, gsl(g)], a[:, :, gsl(g)].bitcast(F32), 0.0,
                    wp[g][:], ALU.is_gt, ALU.mult)

            def v_r(g):
                nc.vector.scalar_tensor_tensor(
                    rr[:, :, gsl(g)], a[:, :, gsl(g)].bitcast(F32), 0.0,
                    up[g][:], ALU.is_gt, ALU.mult)

            def v_rec(g):
                nc.vector.reciprocal_approx_fast(rec[:, gsl(g)], rp[:, gsl(g)])

            def v_a2(g):
                nc.vector.tensor_mul(a2[:, gsl(g)], vs[:, gsl(g)], t2p[g][:])

            def v_cc(g):
                # cc = (t1' * sgn) + a2 = 2 v sig t2 - sgn t1
                nc.vector.scalar_tensor_tensor(
                    cc[:, gsl(g)], t1p[g][:], sgnc[:, 0:1], a2[:, gsl(g)],
                    ALU.mult, ALU.add)

            def tout(g):
                # per-chunk transpose -> copy -> DMA so the final store
                # pipeline drains with 64KB DMAs instead of one 128KB
                otp = pmis.tile([D, 2, D], F32, tag="mis", name="mis")
                for k in range(2):
                    c = 2 * g + k
                    nc.tensor.transpose(otp[:, k, :],
                                        dv[:, c * D:(c + 1) * D], ident[:])
                    nc.scalar.copy(ot[:, c, :], otp[:, k, :])
                    nc.sync.dma_start(
                        out.rearrange("(p c) f -> p c f", c=NCH)
                        [:, c, D:2 * D],
                        ot[:, c, :])

            # ---------------- schedule (priority = program order) ---------
            tin_x(0)
            s_xcopy(0)
            tin_x(1)
            s_xcopy(1)
            mm_h(0)
            tin_v(0)
            v_vcopy(0)
            s_relu(0, 0)
            s_relu(0, 1)
            s_w2prep()
            mm_h(1)
            tin_v(1)
            v_vcopy(1)
            v_nv2()
            v_vsq()
            mm_w(0)
            mm_w(1)
            mm_z(0)
            s_sig(0)
            v_mw(0)
            v_nsig(0)
            g_p(0)
            v_rp(0)
            g_vs(0)
            s_relu(1, 0)
            s_relu(1, 1)
            mm_z(1)
            mm_t2(0)
            s_sig(1)
            mm_u(0)
            v_rec(0)
            v_r(0)
            mm_t1(0)
            v_nsig(1)
            g_p(1)
            v_rp(1)
            g_vs(1)
            v_mw(1)
            v_a2(0)
            v_cc(0)
            mm_t2(1)
            mm_u(1)
            g_dv(0)
            tout(0)
            v_rec(1)
            v_r(1)
            mm_t1(1)
            v_a2(1)
            v_cc(1)
            g_dv(1)
            tout(1)

    return nc


_CACHE = {}


def _get_nc(variant="v4"):
    if variant not in _CACHE:
        nc = bacc.Bacc("TRN2", target_bir_lowering=False, debug=False,
                       num_devices=N_CORES)
        _build(nc)
        nc.compile()
        _CACHE[variant] = nc
    return _CACHE[variant]


def kernel(t, input_, W1, b1, W2, b2):
    input_ = np.ascontiguousarray(np.asarray(input_, dtype=np.float32))
    W1 = np.ascontiguousarray(np.asarray(W1, dtype=np.float32))
    b1 = np.ascontiguousarray(np.asarray(b1, dtype=np.float32))
    W2 = np.ascontiguousarray(np.asarray(W2, dtype=np.float32))
    b2 = np.ascontiguousarray(np.asarray(b2, dtype=np.float32))
    assert input_.shape == (BATCH, 2 * D)

    nc = _get_nc()
    in_maps = [
        {"inp": input_[c * B:(c + 1) * B], "W1": W1, "b1": b1, "W2": W2, "b2": b2}
        for c in range(N_CORES)
    ]
    res = run_bass_kernel_spmd(nc, in_maps, core_ids=list(range(N_CORES)))
    return np.concatenate([res.results[c]["out"] for c in range(N_CORES)], axis=0)


# revision 34
# speedup vs baseline: 1.0259x; 1.0259x over previous
"""Trainium2 Bass kernel for nn_Connection_v5extend (8-core data-parallel).

kernel(**inputs) takes the FULL unsharded inputs (as produced by
setup_inputs) and returns the FULL [4096, 256] float32 output.

Math (per core, B=512 rows, D=128):
    x, v  = input[:, :D], input[:, D:]
    h     = W1 x + b1;  a = relu(h);  mask = (h > 0)
    s     = sigmoid(W2 a + b2);  sig = s (1 - s);  nsig = -sig
    p'    = v^2 * nsig                     (sgn folded into W2s = sgn.W2)
    u'    = W2s^T p' = -W2^T (v^2 sgn sig);  r' = mask * u';  t1' = -W1^T r
    t2    = W2 (mask * (W1 v))
    a2    = (2 v sig) t2;  cc = sgn*t1' + a2 = 2 v sig t2 - sgn t1
    dv    = cc / (s + CONST)
    out   = concat([v, dv], axis=1)

Implementation notes: feature-major on chip (features on partitions,
batch on the free dim); batch rows permuted (row = 4p + c) so each
partition loads one contiguous 2KB block per input DMA (the output
store undoes the permutation); W1 uses a row-parity split (row = 2p+h)
consistently across every k-contraction, which lets W1/W2 load with
large descriptors and b1 spread to partitions with tiny PE transposes;
sgn is folded into the mm_u stationary (w2s = sgn * W2) so no sign
masks appear in the elementwise chain; low-priority warm-up matmuls
ramp the PE clock during the input-DMA wait; the v-passthrough is
written from SBUF after the input lands; elementwise work is spread
across ACT (relu/sigmoid/PSUM drains), DVE (all PSUM-sourced tensor
ops, reciprocal), and Pool/GpSimd (SBUF-only products).
"""

import sys

sys.path.insert(0, "/opt/trn_rl_repo")

import numpy as np

import concourse.bass as bass  # noqa: F401
import concourse.bacc as bacc
import concourse.mybir as mybir
import concourse.tile as tile
from concourse.masks import make_identity
from concourse.bass_utils import run_bass_kernel_spmd

F32 = mybir.dt.float32
F32R = mybir.dt.float32r
AF = mybir.ActivationFunctionType
ALU = mybir.AluOpType

D = 128
CONST = 0.618
SIGN = 4
N_CORES = 8
BATCH = 4096
B = BATCH // N_CORES  # rows per core
NCH = B // D          # 4 batch chunks of 128
SEG = B // 2          # 256 cols per pipeline segment
N_DUM = 2             # warm-up matmuls


def _build(nc):
    inp = nc.dram_tensor("inp", [B, 2 * D], F32, kind="ExternalInput").ap()
    W1 = nc.dram_tensor("W1", [2 * D, D], F32, kind="ExternalInput").ap()
    b1 = nc.dram_tensor("b1", [2 * D], F32, kind="ExternalInput").ap()
    W2 = nc.dram_tensor("W2", [D, 2 * D], F32, kind="ExternalInput").ap()
    b2 = nc.dram_tensor("b2", [D], F32, kind="ExternalInput").ap()
    out = nc.dram_tensor("out", [B, 2 * D], F32, kind="ExternalOutput").ap()

    def gsl(g):
        return slice(g * SEG, (g + 1) * SEG)

    with tile.TileContext(nc) as tc:
        with (
            tc.tile_pool(name="cst", bufs=1) as cst,
            tc.tile_pool(name="sb", bufs=1) as sb,
            tc.tile_pool(name="pblk", bufs=4, space="PSUM") as pblk,
            tc.tile_pool(name="pmis", bufs=1, space="PSUM") as pmis,
            tc.tile_pool(name="pu", bufs=1, space="PSUM") as pu,
            tc.tile_pool(name="pt1", bufs=1, space="PSUM") as pt1,
            tc.tile_pool(name="pt2", bufs=1, space="PSUM") as pt2,
        ):
            # ---------------- DMA issues (first thing on each queue) ------
            it = sb.tile([D, NCH, 2 * D], F32, tag="it", name="it")
            inr = inp.rearrange("(p c) f -> p c f", c=NCH)
            # x columns land first (they gate the transpose->mm_h chain)
            nc.scalar.dma_start(it[:, 0:2, 0:D], inr[:, 0:2, 0:D])
            nc.scalar.dma_start(it[:, 0:2, D:2 * D], inr[:, 0:2, D:2 * D])
            b1row = cst.tile([1, 2 * D], F32, tag="b1row", name="b1row")
            nc.scalar.dma_start(b1row[:], b1.rearrange("(o k) -> o k", o=1))
            b2row = cst.tile([1, D], F32, tag="b2row", name="b2row")
            nc.scalar.dma_start(b2row[:], b2.rearrange("(o k) -> o k", o=1))

            w1k = cst.tile([D, 2, D], F32, tag="w1k", name="w1k")
            nc.sync.dma_start(w1k[:], W1.rearrange("(p h) j -> p h j", h=2))
            nc.sync.dma_start(it[:, 2:4, 0:D], inr[:, 2:4, 0:D])
            nc.sync.dma_start(it[:, 2:4, D:2 * D], inr[:, 2:4, D:2 * D])
            w2n = cst.tile([D, 2 * D], F32, tag="w2n", name="w2n")
            nc.sync.dma_start(w2n[:], W2)
            # v passthrough from SBUF (naturally ordered after input DMAs)
            nc.sync.dma_start(
                out.rearrange("(p c) f -> p c f", c=NCH)[:, :, 0:D],
                it[:, :, D:2 * D])

            # ---------------- constants -----------------------------------
            zer = cst.tile([D, 2 * D], F32, tag="zer", name="zer")
            nc.vector.memset(zer[:], 0.0)
            # sgnc = sgn  (-1 for i < SIGN, +1 otherwise)
            sgnc = cst.tile([D, 1], F32, tag="sgnc", name="sgnc")
            nc.vector.memset(sgnc[:], 1.0)
            nc.vector.memset(sgnc[:SIGN, :], -1.0)
            dum = cst.tile([D, 2 * D], F32R, tag="dum", name="dum")
            nc.vector.tensor_copy(dum[:], zer[:])

            ident = cst.tile([D, D], F32, tag="ident", name="ident")
            make_identity(nc, ident[:])

            # prime the {sigmoid,relu,identity,copy} activation table set
            warmt = cst.tile([D, 1], F32, tag="warmt", name="warmt")
            nc.scalar.activation(warmt[:], sgnc[:, 0:1], AF.Sigmoid)

            # ---------------- bias spread (tiny PE transposes) ------------
            # b1c[p, h] = b1[2p + h] matching the W1 row-parity split
            bias_ps = pmis.tile([D, 3], F32, tag="mis", name="mis")
            b1v = b1row[0:1, :].rearrange("o (p h) -> o h p", h=2)
            nc.tensor.transpose(bias_ps[:, 0:1], b1v[:, 0, :], ident[0:1, 0:1])
            nc.tensor.transpose(bias_ps[:, 1:2], b1v[:, 1, :], ident[0:1, 0:1])
            nc.tensor.transpose(bias_ps[:, 2:3], b2row[0:1, :],
                                ident[0:1, 0:1])
            bcols = cst.tile([D, 3], F32, tag="bcols", name="bcols")
            nc.vector.tensor_copy(bcols[:], bias_ps[:])

            # ---------------- weight prep ---------------------------------
            # w1T[:, h, :] = W1 parity-half h, transposed (PE + scalar copy)
            w1T = cst.tile([D, 2, D], F32R, tag="w1T", name="w1T")
            w1T_ps = pmis.tile([D, 2, D], F32, tag="mis", name="mis")
            nc.tensor.transpose(w1T_ps[:, 0, :], w1k[:, 0, :], ident[:])
            nc.tensor.transpose(w1T_ps[:, 1, :], w1k[:, 1, :], ident[:])
            nc.scalar.copy(w1T[:], w1T_ps[:])

            # w2s = sgn-row-scaled W2 (f32r), stationary for mm_u
            # (the scalar op is issued later, after the first relus)
            w2s = cst.tile([D, 2 * D], F32R, tag="w2s", name="w2s")
            w2ur = w2s[:].rearrange("i (k h) -> i h k", h=2)

            # w1kr: f32r copy of W1 (stationary for mm_t1)
            w1kr = cst.tile([D, 2, D], F32R, tag="w1kr", name="w1kr")
            nc.vector.tensor_copy(w1kr[:], w1k[:])

            # w2T[:, h, :] = transpose of W2 columns with parity h
            w2v = w2n[:].rearrange("i (k h) -> i h k", h=2)
            w2T = cst.tile([D, 2, D], F32R, tag="w2T", name="w2T")
            w2T_ps = pmis.tile([D, 2, D], F32, tag="mis", name="mis")
            nc.tensor.transpose(w2T_ps[:, 0, :], w2v[:, 0, :], ident[:])
            nc.tensor.transpose(w2T_ps[:, 1, :], w2v[:, 1, :], ident[:])

            def s_w2prep():
                nc.scalar.activation(w2s[:], w2n[:], AF.Copy,
                                     scale=sgnc[:, 0:1])
                nc.scalar.copy(w2T[:], w2T_ps[:])

            # ---------------- PE warm-up ----------------------------------
            # independent matmuls on zeros keep the PE busy (HAM clock ramp)
            # during the input-DMA wait; low priority so real PE work
            # preempts them as its inputs land.  The final copy into dum
            # (value-preserving: 0.0) gives pd a reader so DCE keeps them.
            pd = pu.tile([D, 2 * D], F32, tag="u", name="u")
            for i in range(N_DUM):
                nc.tensor.matmul(pd[:], dum[:, 0:D], dum[:])
            nc.vector.tensor_copy(dum[:, 0:1], pd[:, 0:1])

            # ---------------- data tiles ----------------------------------
            # xv[:, 0, c, :] = x chunk c transposed; [:, 1, c, :] = v
            xv = sb.tile([D, 2, NCH, D], F32R, tag="xv", name="xv")
            a = sb.tile([D, 2, B], F32R, tag="a", name="a")
            mw = sb.tile([D, 2, B], F32R, tag="mwt", name="mwt")
            rr = sb.tile([D, 2, B], F32R, tag="rr", name="rr")
            s_ = sb.tile([D, B], F32, tag="s", name="s")
            nsig = sb.tile([D, B], F32, tag="nsig", name="nsig")
            vsq = sb.tile([D, B], F32, tag="vsq", name="vsq")
            p_ = sb.tile([D, B], F32R, tag="p", name="p")
            rp = sb.tile([D, B], F32, tag="rp", name="rp")
            rec = sb.tile([D, B], F32, tag="rec", name="rec")
            vs = sb.tile([D, B], F32, tag="vs", name="vs")
            nv2 = sb.tile([D, B], F32, tag="nv2", name="nv2")
            a2 = sb.tile([D, B], F32, tag="a2", name="a2")
            cc = sb.tile([D, B], F32, tag="cc", name="cc")
            dv = sb.tile([D, B], F32, tag="dv", name="dv")
            ot = sb.tile([D, NCH, D], F32, tag="ot", name="ot")

            vTf = xv[:, 1, :, :].rearrange("p c d -> p (c d)").bitcast(F32)

            pxv = [None, None]
            hp = [None, None]
            wp = [None, None]
            zp = [None, None]
            up = [None, None]
            t1p = [None, None]
            t2p = [None, None]

            # ---------------- stage helpers -------------------------------
            def tin_x(g):
                pxv[g] = pblk.tile([D, 2, 2, D], F32, tag="blk", name="blk")
                for k in range(2):
                    c = 2 * g + k
                    nc.tensor.transpose(pxv[g][:, 0, k, :], it[:, c, 0:D],
                                        ident[:])

            def tin_v(g):
                for k in range(2):
                    c = 2 * g + k
                    nc.tensor.transpose(pxv[g][:, 1, k, :], it[:, c, D:2 * D],
                                        ident[:])

            def s_xcopy(g):
                nc.scalar.copy(xv[:, 0, 2 * g:2 * g + 2, :], pxv[g][:, 0, :, :])

            def v_vcopy(g):
                nc.vector.tensor_copy(xv[:, 1, 2 * g:2 * g + 2, :],
                                      pxv[g][:, 1, :, :])

            def xT(g):
                return xv[:, 0, 2 * g:2 * g + 2, :].rearrange(
                    "p c d -> p (c d)")

            def vT(g):
                return xv[:, 1, 2 * g:2 * g + 2, :].rearrange(
                    "p c d -> p (c d)")

            def mm_h(g):
                hp[g] = pblk.tile([D, 2, SEG], F32, tag="blk", name="blk")
                nc.tensor.matmul(hp[g][:, 0, :], w1T[:, 0, :], xT(g))
                nc.tensor.matmul(hp[g][:, 1, :], w1T[:, 1, :], xT(g))

            def mm_w(g):
                wp[g] = pblk.tile([D, 2, SEG], F32, tag="blk", name="blk")
                nc.tensor.matmul(wp[g][:, 0, :], w1T[:, 0, :], vT(g))
                nc.tensor.matmul(wp[g][:, 1, :], w1T[:, 1, :], vT(g))

            def mm_z(g):
                zp[g] = pmis.tile([D, SEG], F32, tag="mis", name="mis")
                nc.tensor.matmul(zp[g][:], w2T[:, 0, :], a[:, 0, gsl(g)],
                                 start=True, stop=False)
                nc.tensor.matmul(zp[g][:], w2T[:, 1, :], a[:, 1, gsl(g)],
                                 start=False, stop=True)

            def mm_u(g):
                up[g] = pu.tile([D, 2, SEG], F32, tag="u", name="u")
                nc.tensor.matmul(up[g][:, 0, :], w2ur[:, 0, :], p_[:, gsl(g)])
                nc.tensor.matmul(up[g][:, 1, :], w2ur[:, 1, :], p_[:, gsl(g)])

            def mm_t1(g):
                t1p[g] = pt1.tile([D, SEG], F32, tag="t1", name="t1")
                nc.tensor.matmul(t1p[g][:], w1kr[:, 0, :], rr[:, 0, gsl(g)],
                                 start=True, stop=False)
                nc.tensor.matmul(t1p[g][:], w1kr[:, 1, :], rr[:, 1, gsl(g)],
                                 start=False, stop=True)

            def mm_t2(g):
                t2p[g] = pt2.tile([D, SEG], F32, tag="t2", name="t2")
                nc.tensor.matmul(t2p[g][:], w2T[:, 0, :], mw[:, 0, gsl(g)],
                                 start=True, stop=False)
                nc.tensor.matmul(t2p[g][:], w2T[:, 1, :], mw[:, 1, gsl(g)],
                                 start=False, stop=True)

            # elementwise stages
            def s_relu(g, h):
                nc.scalar.activation(a[:, h, gsl(g)], hp[g][:, h, :], AF.Relu,
                                     bias=bcols[:, h:h + 1])

            def s_sig(g):
                nc.scalar.activation(s_[:, gsl(g)], zp[g][:], AF.Sigmoid,
                                     bias=bcols[:, 2:3])

            def v_nv2():
                # nv2 = -2v
                nc.vector.tensor_single_scalar(nv2[:], vTf, -2.0, ALU.mult)

            def v_vsq():
                nc.vector.tensor_mul(vsq[:], vTf, vTf)

            def v_nsig(g):
                # nsig = (s - 1) * s = -s(1-s)
                nc.vector.scalar_tensor_tensor(
                    nsig[:, gsl(g)], s_[:, gsl(g)], 1.0, s_[:, gsl(g)],
                    ALU.subtract, ALU.mult)

            def g_p(g):
                # p' = v^2 * nsig = -v^2 sig  (sgn lives in w2s)
                nc.gpsimd.tensor_mul(p_[:, gsl(g)], vsq[:, gsl(g)],
                                     nsig[:, gsl(g)])

            def v_rp(g):
                nc.vector.tensor_single_scalar(
                    rp[:, gsl(g)], s_[:, gsl(g)], CONST, ALU.add)

            def g_vs(g):
                # vs = (-2v) * nsig = 2 v sig
                nc.gpsimd.tensor_mul(vs[:, gsl(g)], nv2[:, gsl(g)],
                                     nsig[:, gsl(g)])

            def g_dv(g):
                nc.gpsimd.tensor_mul(dv[:, gsl(g)], cc[:, gsl(g)],
                                     rec[:, gsl(g)])

            def v_mw(g):
                # mw = (a > 0) * (W1 v)
                nc.vector.scalar_tensor_tensor(
                    mw[:, :, gsl(g)], a[:, :, gsl(g)].bitcast(F32), 0.0,
                    wp[g][:], ALU.is_gt, ALU.mult)

            def v_r(g):
                nc.vector.scalar_tensor_tensor(
                    rr[:, :, gsl(g)], a[:, :, gsl(g)].bitcast(F32), 0.0,
                    up[g][:], ALU.is_gt, ALU.mult)

            def v_rec(g):
                nc.vector.reciprocal_approx_fast(rec[:, gsl(g)], rp[:, gsl(g)])

            def v_a2(g):
                nc.vector.tensor_mul(a2[:, gsl(g)], vs[:, gsl(g)], t2p[g][:])

            def v_cc(g):
                # cc = (t1' * sgn) + a2 = 2 v sig t2 - sgn t1
                nc.vector.scalar_tensor_tensor(
                    cc[:, gsl(g)], t1p[g][:], sgnc[:, 0:1], a2[:, gsl(g)],
                    ALU.mult, ALU.add)

            def tout(g):
                # per-chunk transpose -> copy -> DMA so the final store
                # pipeline drains with 64KB DMAs instead of one 128KB
                otp = pmis.tile([D, 2, D], F32, tag="mis", name="mis")
                for k in range(2):
                    c = 2 * g + k
                    nc.tensor.transpose(otp[:, k, :],
                                        dv[:, c * D:(c + 1) * D], ident[:])
                    nc.scalar.copy(ot[:, c, :], otp[:, k, :])
                    nc.sync.dma_start(
                        out.rearrange("(p c) f -> p c f", c=NCH)
                        [:, c, D:2 * D],
                        ot[:, c, :])

            # ---------------- schedule (priority = program order) ---------
            tin_x(0)
            s_xcopy(0)
            tin_x(1)
            s_xcopy(1)
            mm_h(0)
            tin_v(0)
            v_vcopy(0)
            s_relu(0, 0)
            s_relu(0, 1)
            s_w2prep()
            mm_h(1)
            tin_v(1)
            v_vcopy(1)
            v_nv2()
            v_vsq()
            mm_w(0)
            mm_w(1)
            mm_z(0)
            s_sig(0)
            v_mw(0)
            v_nsig(0)
            g_p(0)
            v_rp(0)
            g_vs(0)
            s_relu(1, 0)
            s_relu(1, 1)
            mm_z(1)
            mm_t2(0)
            s_sig(1)
            mm_u(0)
            v_rec(0)
            v_r(0)
            mm_t1(0)
            v_nsig(1)
            g_p(1)
            v_rp(1)
            g_vs(1)
            v_mw(1)
            v_a2(0)
            v_cc(0)
            mm_t2(1)
            mm_u(1)
            g_dv(0)
            tout(0)
            v_rec(1)
            v_r(1)
            mm_t1(1)
            v_a2(1)
            v_cc(1)
            g_dv(1)
            tout(1)

    return nc


_CACHE = {}


def _get_nc(variant="v4"):
    if variant not in _CACHE:
        nc = bacc.Bacc("TRN2", target_bir_lowering=False, debug=False,
                       num_devices=N_CORES)
        _build(nc)
        nc.compile()
        _CACHE[variant] = nc
    return _CACHE[variant]


def kernel(t, input_, W1, b1, W2, b2):
    input_ = np.ascontiguousarray(np.asarray(input_, dtype=np.float32))
    W1 = np.ascontiguousarray(np.asarray(W1, dtype=np.float32))
    b1 = np.ascontiguousarray(np.asarray(b1, dtype=np.float32))
    W2 = np.ascontiguousarray(np.asarray(W2, dtype=np.float32))
    b2 = np.ascontiguousarray(np.asarray(b2, dtype=np.float32))
    assert input_.shape == (BATCH, 2 * D)

    nc = _get_nc()
    in_maps = [
        {"inp": input_[c * B:(c + 1) * B], "W1": W1, "b1": b1, "W2": W2, "b2": b2}
        for c in range(N_CORES)
    ]
    res = run_bass_kernel_spmd(nc, in_maps, core_ids=list(range(N_CORES)))
    return np.concatenate([res.results[c]["out"] for c in range(N_CORES)], axis=0)
